# revision 39
# baseline (speedup 1.0000x reference)
"""Trainium2 Bass kernel for nn_Deep_AD_F_58213986730479 (dense_cnn).

Math (per iteration t of 3):
    feats = 4 one-pixel zero-padded shifts (N,S,W,E) of x        [n,4,h,w]
    d     = conv3x3(feats, W[t]) + b[t]                          [n,4,h,w]
    x    -= sum_k d_k * exp(-d_k^2) / 4

Implementation:
  - Pure data parallel: batch 32 -> 8 cores x 4 images.
  - The shift+conv composes into a 21-tap stencil on x. Vertical taps are
    applied with banded-matrix matmuls on TensorE (contraction over image
    rows on partitions); horizontal taps via 5 column-shifted accumulating
    matmuls into PSUM. Boundary semantics of the double zero-padding are
    exact: row-edge terms fold into per-tile band-matrix variants; column
    edge terms are two N=1 correction matmuls per channel.
  - exp(-d^2) comes from one ScalarE op: Derivative_Erf = 2/sqrt(pi)*exp(-x^2);
    the bias add (d+b) rides free in the activation and in the DVE
    scalar_tensor_tensor that forms gated = (d+b)*e. Channel sum split
    GpSimd (s01,s23) + DVE (stot); x update is one fused DVE STT.
  - Each 512x512 image is 5 row-tiles [128,512] (stride 116, 6-row halo);
    3 iterations shrink the valid halo by 2 rows each, so no cross-tile
    traffic is ever needed.
  - All matmuls run in bf16 (4x the fp32 rate: fp32 = 2 hi/lo passes at
    half stream rate); x is held in bf16 in SBUF so the PE reads it
    directly (no per-step copies). Loads stage fp32->bf16 via DVE; the
    final iteration's update writes fp32 straight to store staging and
    DMAs out (no bf16 roundtrip). Weight-table DMA is chunked per (t,k)
    and interleaved with image loads so the first matmul starts early.
    End-to-end rel err ~3e-3 (bf16 state rounding), HW ~335us vs 1186us
    for the all-fp32 version of the same structure.
"""
import sys

sys.path.insert(0, "/opt/trn_rl_repo")

import math
import numpy as np

import concourse.bass as bass
import concourse.bacc as bacc
import concourse.mybir as mybir
from concourse.tile import TileContext
from concourse.bass_utils import run_bass_kernel_spmd

F32 = mybir.dt.float32
F32R = mybir.dt.float32r
BF16 = mybir.dt.bfloat16
AF = mybir.ActivationFunctionType
ALU = mybir.AluOpType

NCORES = 8
IMGS = 4          # images per core
H = W_IMG = 512
T_ITERS = 3
KCH = 4
NTILES = 5
TSTART = [-6, 110, 226, 342, 458]   # image row held by partition 0 of tile j
CORE_LO = 6                          # first owned partition of each tile
CORE_ROWS = [116, 116, 116, 116, 48]
C_UPD = math.sqrt(math.pi) / 8.0     # 1/4 * sqrt(pi)/2 (Derivative_Erf scale)

# feats channel order in reference: N, S, W, E
OY = [-1, 1, 0, 0]
OX = [0, 0, -1, 1]

DXS = [0, -1, 1, -2, 2]

# debug bisect flags
_SKIP_CORR = __import__("os").environ.get("KERNEL_SKIP_CORR", "0") == "1"
_PSUM_BUFS = 2
_INPLACE_UPD = True
_MASK_AP = True
_TILE_SET = None  # e.g. [2] to restrict tiles (debug)
_PAD_BMAT = True
_MM_DTYPE = __import__("os").environ.get("KERNEL_MM_DTYPE", "bf16")  # f32 | f32r | bf16
_BISECT = __import__("os").environ.get("KERNEL_BISECT", "")  # comma list: updf32,gpstot,ldscalar,stscalar
_XBF16 = __import__("os").environ.get("KERNEL_XBF16", "1") == "1"
_SIMPLE_BIAS = False  # Dx=0 first: full-range start=True write


def _composite_taps(Wc):
    """T[t,k,Dy+2,Dx+2] = sum of W[t,k,i,dy+1,dx+1] with dy+oy_i=Dy, dx+ox_i=Dx."""
    taps = np.zeros((T_ITERS, KCH, 5, 5), np.float64)
    for t in range(T_ITERS):
        for k in range(KCH):
            for i in range(4):
                for dy in (-1, 0, 1):
                    for dx in (-1, 0, 1):
                        taps[t, k, dy + OY[i] + 2, dx + OX[i] + 2] += Wc[
                            t, k, i, dy + 1, dx + 1
                        ]
    return taps


def _build_bmats(Wc):
    """Dense lhsT matrices, returned as array [NB,128,128] f32 plus an index fn.

    Layout per (t,k): 5 generic B_Dx, then 3 top-variant (Dx=-1,0,1), then
    3 bottom-variant, then left corr, right corr = 13 matrices.
    B[in_row, out_row] = tap[in-out, Dx].
    """
    taps = _composite_taps(Wc)
    mats = []
    index = {}

    def band(vals_by_dy):
        B = np.zeros((128, 128), np.float64)
        for dy, v in vals_by_dy.items():
            B += v * np.eye(128, k=-dy)
        return B

    for t in range(T_ITERS):
        for k in range(KCH):
            per_dx = {}
            for Dx in (-2, -1, 0, 1, 2):
                per_dx[Dx] = band(
                    {Dy: taps[t, k, Dy + 2, Dx + 2] for Dy in range(-2, 3)}
                )
            for Dx in DXS:
                index[(t, k, Dx, "mid")] = len(mats)
                mats.append(per_dx[Dx])
            for Dx in (-1, 0, 1):
                Btop = per_dx[Dx].copy()
                # image row 0 = partition CORE_LO of tile 0: remove south-ch dy=-1
                Btop[CORE_LO, CORE_LO] -= Wc[t, k, 1, 0, Dx + 1]
                index[(t, k, Dx, "top")] = len(mats)
                mats.append(Btop)
            for Dx in (-1, 0, 1):
                Bbot = per_dx[Dx].copy()
                # image row 511 = partition 53 of tile 4: remove north-ch dy=+1
                p = CORE_LO + (H - 1) - TSTART[4] - CORE_LO  # = 53
                Bbot[p, p] -= Wc[t, k, 0, 2, Dx + 1]
                index[(t, k, Dx, "bot")] = len(mats)
                mats.append(Bbot)
            # column-edge corrections (vertical 3-tap bands)
            Bl = band({dy: -Wc[t, k, 3, dy + 1, 0] for dy in (-1, 0, 1)})
            index[(t, k, "corrL")] = len(mats)
            mats.append(Bl)
            Br = band({dy: -Wc[t, k, 2, dy + 1, 2] for dy in (-1, 0, 1)})
            index[(t, k, "corrR")] = len(mats)
            mats.append(Br)
    # tile-4 packed pair matrices: 2 channels stacked in out cols
    # (k at cols 64*blk + r, r in 0..47, image row 464+r); bot variant baked in
    def p4_row(m):
        return (0, m) if m < 48 else ((1, m - 64) if 64 <= m < 112 else None)

    for t in range(T_ITERS):
        for pair in range(2):
            for Dx in DXS:
                B = np.zeros((128, 128), np.float64)
                for m in range(128):
                    kr = p4_row(m)
                    if kr is None:
                        continue
                    kk, r = 2 * pair + kr[0], kr[1]
                    for p in range(54):
                        Dy = p - r - 6
                        if -2 <= Dy <= 2:
                            B[p, m] = taps[t, kk, Dy + 2, Dx + 2]
                    if r == 47 and -1 <= Dx <= 1:  # image row 511: remove north dy=+1
                        B[53, m] -= Wc[t, kk, 0, 2, Dx + 1]
                index[(t, pair, Dx, "p4")] = len(mats)
                mats.append(B)
            for name, ch, col in (("corrL", 3, 0), ("corrR", 2, 2)):
                B = np.zeros((128, 128), np.float64)
                for m in range(128):
                    kr = p4_row(m)
                    if kr is None:
                        continue
                    kk, r = 2 * pair + kr[0], kr[1]
                    for Dy in (-1, 0, 1):
                        p = r + 6 + Dy
                        if 0 <= p < 54:
                            B[p, m] = -Wc[t, kk, ch, Dy + 1, col]
                index[(t, pair, name, "p4")] = len(mats)
                mats.append(B)
    Sel = np.zeros((128, 128), np.float64)
    for r in range(48):
        Sel[r, 6 + r] = 1.0
        Sel[64 + r, 6 + r] = 1.0
    index[("sel4",)] = len(mats)
    mats.append(Sel)
    arr = np.stack(mats).astype(np.float32)
    return arr, index


_NB = T_ITERS * KCH * 13 + T_ITERS * 2 * 7 + 1


def _build_masks():
    """Per-tile update masks [128, NTILES]: -C_UPD at real image rows, 0 at pad."""
    m = np.full((128, NTILES), -C_UPD, np.float32)
    for j in range(NTILES):
        r0 = TSTART[j]
        plo = max(0, -r0)
        phi = min(128, H - r0)
        m[0:plo, j] = 0.0
        m[phi:128, j] = 0.0
    return m


def _build_nc(bvals, bindex):
    nc = bacc.Bacc(None, target_bir_lowering=False)
    xs = nc.declare_dram_parameter("xs", [IMGS, H, W_IMG], F32, isOutput=False)
    bmw = _NB * 128 + (0 if _MM_DTYPE == "bf16" else NTILES)
    if _PAD_BMAT:
        bmw = (bmw + 127) // 128 * 128
    bdt = BF16 if _MM_DTYPE == "bf16" else F32
    bm = nc.declare_dram_parameter("bmat", [128, bmw], bdt, isOutput=False)
    if _MM_DTYPE == "bf16":
        aux = nc.declare_dram_parameter("aux", [128, 16], F32, isOutput=False)
    yo = nc.declare_dram_parameter("out", [IMGS, H, W_IMG], F32, isOutput=True)

    with TileContext(nc) as tc:
        with (
            tc.tile_pool(name="wts", bufs=1) as wp,
            tc.tile_pool(name="xdata", bufs=1) as xp,
            tc.tile_pool(name="work", bufs=3) as sp,
            tc.tile_pool(name="ps", bufs=2, space="PSUM") as pp,
        ):
            bmt = wp.tile([128, bmw], bdt, tag="bmt")
            pert = KCH * 13 * 128  # cols per iteration t
            pk = 13 * 128  # cols per (t, k) chunk
            if _MM_DTYPE == "bf16":
                auxt = wp.tile([128, 16], F32, tag="auxt")
                nc.gpsimd.dma_start(out=auxt[:], in_=aux[:])

            def load_bmat_chunk(t, k):
                if t == "p4":
                    c0 = (T_ITERS * KCH * 13 + k * 14) * 128
                    cw = 14 * 128 + (128 if k == T_ITERS - 1 else 0)  # +selector
                else:
                    c0 = t * pert + k * pk
                    cw = pk
                nc.sync.dma_start(out=bmt[:, c0 : c0 + cw], in_=bm[:, c0 : c0 + cw])

            if _SIMPLE_BIAS:
                bias_tiles = {}
                for t in range(T_ITERS):
                    for k in range(KCH):
                        bb = wp.tile([128, 1], F32, tag=f"bias{t}_{k}")
                        nc.vector.memset(bb[:], float(bvals[t, k]))
                        bias_tiles[t, k] = bb
            else:
                bias_t = wp.tile([128, T_ITERS * KCH], F32, tag="bias")
                for t in range(T_ITERS):
                    for k in range(KCH):
                        nc.vector.memset(
                            bias_t[:, t * KCH + k : t * KCH + k + 1], float(bvals[t, k])
                        )

            def bmat(idx):
                ap = bmt[:, idx * 128 : (idx + 1) * 128]
                if _MM_DTYPE == "f32r":
                    ap = ap.bitcast(F32R)
                return ap

            def mm_rhs(ap):
                if _MM_DTYPE == "f32r":
                    return ap.bitcast(F32R)
                return ap

            tset = _TILE_SET if _TILE_SET is not None else list(range(NTILES))
            xbf = _MM_DTYPE == "bf16" and _XBF16
            xdt = BF16 if xbf else F32
            xt = {}
            # interleave the weight-table chunks with the x loads so the first
            # matmuls (needing t=0 bands + im0 tiles) start early; only the
            # sync-engine DMA ring is fast, so everything stays on it
            bm_sched = {  # (im, j) -> list of (t, k) chunks to enqueue after
                (0, 0): [(0, 0), (0, 1), (0, 2), (0, 3)],
                (0, 1): [("p4", 0)],
                (0, 4): [(1, 0), (1, 1)],
                (1, 0): [(1, 2), (1, 3)], (1, 1): [(2, 0), (2, 1), ("p4", 1)],
                (1, 2): [(2, 2), (2, 3), ("p4", 2)],
            }
            for im in range(IMGS):
                for j in tset:
                    tile = xp.tile([128, W_IMG], xdt, tag=f"x{im}_{j}")
                    xt[im, j] = tile
                    r0 = TSTART[j]
                    plo = max(0, -r0)
                    phi = min(128, H - r0)
                    if plo > 0 or phi < 128:
                        nc.vector.memset(tile[:], 0.0)
                    if xbf:
                        stg = sp.tile([128, W_IMG], F32, tag="ldstage")
                        if plo > 0 or phi < 128:
                            nc.vector.memset(stg[:], 0.0)
                        nc.sync.dma_start(
                            out=stg[plo:phi, :], in_=xs[im, r0 + plo : r0 + phi, :]
                        )
                        nc.vector.tensor_copy(tile[:], stg[:])
                    else:
                        nc.sync.dma_start(
                            out=tile[plo:phi, :], in_=xs[im, r0 + plo : r0 + phi, :]
                        )
                    for tk in bm_sched.get((im, j), []):
                        load_bmat_chunk(*tk)
            if bmw > T_ITERS * pert:
                nc.sync.dma_start(
                    out=bmt[:, T_ITERS * pert :], in_=bm[:, T_ITERS * pert :]
                )

            pending_p4 = [None]

            def flush_p4():
                if pending_p4[0] is not None:
                    pending_p4[0]()
                    pending_p4[0] = None

            for it in range(T_ITERS):
                for im in range(IMGS):
                    for j in tset:
                        x_t = xt[im, j]
                        cls = "top" if j == 0 else ("bot" if j == NTILES - 1 else "mid")
                        if _MM_DTYPE == "bf16" and not xbf:
                            xmm = sp.tile([128, W_IMG], BF16, tag="xb")
                            nc.scalar.copy(xmm[:], x_t[:])
                        else:
                            xmm = x_t
                        if j == NTILES - 1 and _MM_DTYPE == "bf16":
                            # tile 4: 48 owned rows -> 2 channels packed per MM
                            dA = pp.tile([128, W_IMG], F32, tag="d0")
                            dB = pp.tile([128, W_IMG], F32, tag="d1")
                            for pair, dP in ((0, dA), (1, dB)):
                                for Dx in DXS:
                                    ocl = max(0, -Dx)
                                    och = W_IMG - max(0, Dx)
                                    nc.tensor.matmul(
                                        dP[:, ocl:och],
                                        bmat(bindex[(it, pair, Dx, "p4")]),
                                        mm_rhs(xmm[:, ocl + Dx : och + Dx]),
                                        start=(Dx == 0),
                                        stop=False,
                                    )
                                nc.tensor.matmul(
                                    dP[:, 0:1],
                                    bmat(bindex[(it, pair, "corrL", "p4")]),
                                    mm_rhs(xmm[:, 0:1]),
                                    start=False, stop=False,
                                )
                                nc.tensor.matmul(
                                    dP[:, W_IMG - 1 : W_IMG],
                                    bmat(bindex[(it, pair, "corrR", "p4")]),
                                    mm_rhs(xmm[:, W_IMG - 1 : W_IMG]),
                                    start=False, stop=True,
                                )
                            flush_p4()
                            stot4 = pp.tile([128, W_IMG], F32, tag="d2")
                            gPs = []
                            for pair, dP in ((0, dA), (1, dB)):
                                bcol = 5 + it * 2 + pair
                                eP = sp.tile([128, W_IMG], F32, tag=f"e{pair}")
                                nc.scalar.activation(
                                    eP[:], dP[:], AF.Derivative_Erf,
                                    bias=auxt[:, bcol : bcol + 1], scale=1.0,
                                )
                                gP = sp.tile([128, W_IMG], BF16, tag=f"g4{pair}")
                                nc.vector.scalar_tensor_tensor(
                                    out=gP[:], in0=dP[:],
                                    scalar=auxt[:, bcol : bcol + 1], in1=eP[:],
                                    op0=ALU.add, op1=ALU.mult,
                                )
                                gPs.append(gP)

                            def _p4_tail(it=it, im=im, j=j, x_t=x_t,
                                         stot4=stot4, gPs=gPs):
                                # deferred: selector MMs run after the NEXT
                                # step's stencil MMs so the PE queue never
                                # stalls waiting on the DVE-produced g tiles
                                for pair in range(2):
                                    nc.tensor.matmul(
                                        stot4[:], bmat(bindex[("sel4",)]),
                                        gPs[pair][:],
                                        start=(pair == 0), stop=(pair == 1),
                                    )
                                mask4 = auxt[:, j : j + 1] if _MASK_AP else -C_UPD
                                if it == T_ITERS - 1:
                                    rows = CORE_ROWS[j]
                                    stg = sp.tile([128, W_IMG], F32, tag="ststage")
                                    nc.vector.scalar_tensor_tensor(
                                        out=stg[:], in0=stot4[:], scalar=mask4,
                                        in1=x_t[:], op0=ALU.mult, op1=ALU.add,
                                    )
                                    nc.sync.dma_start(
                                        out=yo[im, 116 * j : 116 * j + rows, :],
                                        in_=stg[CORE_LO : CORE_LO + rows, :],
                                    )
                                else:
                                    nc.vector.scalar_tensor_tensor(
                                        out=x_t[:], in0=stot4[:], scalar=mask4,
                                        in1=x_t[:], op0=ALU.mult, op1=ALU.add,
                                    )

                            pending_p4[0] = _p4_tail
                            continue
                        dks = []
                        for k in range(KCH):
                            dk_t = pp.tile([128, W_IMG], F32, tag=f"d{k}")
                            dks.append(dk_t)
                        for k in range(KCH):
                            base = 0
                            d = dks[k]
                            for Dx in DXS:
                                key = (
                                    (it, k, Dx, cls)
                                    if (it, k, Dx, cls) in bindex
                                    else (it, k, Dx, "mid")
                                )
                                ocl = max(0, -Dx)
                                och = W_IMG - max(0, Dx)
                                nc.tensor.matmul(
                                    d[:, base + ocl : base + och],
                                    bmat(bindex[key]),
                                    mm_rhs(xmm[:, ocl + Dx : och + Dx]),
                                    start=(Dx == 0),
                                    stop=(_SKIP_CORR and Dx == DXS[-1]),
                                )
                            if not _SKIP_CORR:
                                nc.tensor.matmul(
                                    d[:, base : base + 1],
                                    bmat(bindex[(it, k, "corrL")]),
                                    mm_rhs(xmm[:, 0:1]),
                                    start=False,
                                    stop=False,
                                )
                                nc.tensor.matmul(
                                    d[:, base + W_IMG - 1 : base + W_IMG],
                                    bmat(bindex[(it, k, "corrR")]),
                                    mm_rhs(xmm[:, W_IMG - 1 : W_IMG]),
                                    start=False,
                                    stop=True,
                                )
                        flush_p4()
                        g = sp.tile([128, KCH * W_IMG], F32, tag="g")
                        for k in range(KCH):
                            base = k * W_IMG
                            ek = sp.tile([128, W_IMG], F32, tag=f"e{k}")
                            nc.scalar.activation(
                                ek[:],
                                dks[k][:],
                                AF.Derivative_Erf,
                                bias=(bias_tiles[it, k][:, 0:1] if _SIMPLE_BIAS
                                      else bias_t[:, it * KCH + k : it * KCH + k + 1]),
                                scale=1.0,
                            )
                            nc.vector.scalar_tensor_tensor(
                                out=g[:, base : base + W_IMG],
                                in0=dks[k][:],
                                scalar=float(bvals[it, k]),
                                in1=ek[:],
                                op0=ALU.add,
                                op1=ALU.mult,
                            )
                        s01 = sp.tile([128, W_IMG], F32, tag="s01")
                        s23 = sp.tile([128, W_IMG], F32, tag="s23")
                        stot = sp.tile([128, W_IMG], F32, tag="stot")
                        nc.gpsimd.tensor_tensor(
                            out=s01[:], in0=g[:, 0:512], in1=g[:, 512:1024], op=ALU.add
                        )
                        nc.gpsimd.tensor_tensor(
                            out=s23[:], in0=g[:, 1024:1536], in1=g[:, 1536:2048],
                            op=ALU.add,
                        )
                        if "gpstot" in _BISECT or it == T_ITERS - 1:
                            # final iteration is DVE-bound (store-path update
                            # added); GpSimd has headroom there
                            nc.gpsimd.tensor_tensor(
                                out=stot[:], in0=s01[:], in1=s23[:], op=ALU.add
                            )
                        else:
                            nc.vector.tensor_tensor(
                                out=stot[:], in0=s01[:], in1=s23[:], op=ALU.add
                            )
                        if _MM_DTYPE == "bf16":
                            mask_ap = auxt[:, j : j + 1] if _MASK_AP else -C_UPD
                        else:
                            mask_ap = (
                                bmt[:, _NB * 128 + j : _NB * 128 + j + 1]
                                if _MASK_AP
                                else -C_UPD
                            )
                        if it == T_ITERS - 1:
                            # final update: write fp32 directly to store staging
                            # and DMA out; x_t is never read again
                            rows = CORE_ROWS[j]
                            stg = sp.tile([128, W_IMG], F32, tag="ststage")
                            nc.vector.scalar_tensor_tensor(
                                out=stg[:],
                                in0=stot[:],
                                scalar=mask_ap,
                                in1=x_t[:],
                                op0=ALU.mult,
                                op1=ALU.add,
                            )
                            nc.sync.dma_start(
                                out=yo[im, 116 * j : 116 * j + rows, :],
                                in_=stg[CORE_LO : CORE_LO + rows, :],
                            )
                        elif "updf32" in _BISECT and xbf:
                            xn32 = sp.tile([128, W_IMG], F32, tag="xn32")
                            nc.vector.scalar_tensor_tensor(
                                out=xn32[:],
                                in0=stot[:],
                                scalar=mask_ap,
                                in1=x_t[:],
                                op0=ALU.mult,
                                op1=ALU.add,
                            )
                            nc.vector.tensor_copy(x_t[:], xn32[:])
                        elif _INPLACE_UPD:
                            nc.vector.scalar_tensor_tensor(
                                out=x_t[:],
                                in0=stot[:],
                                scalar=mask_ap,
                                in1=x_t[:],
                                op0=ALU.mult,
                                op1=ALU.add,
                            )
                        else:
                            x_new = xp.tile([128, W_IMG], F32, tag=f"xn{im}_{j}_{it}")
                            nc.vector.scalar_tensor_tensor(
                                out=x_new[:],
                                in0=stot[:],
                                scalar=mask_ap,
                                in1=x_t[:],
                                op0=ALU.mult,
                                op1=ALU.add,
                            )
                            xt[im, j] = x_new

            flush_p4()
    nc.compile()
    return nc


_CACHE = {}


def _get_program(Wc, bc):
    key = (Wc.tobytes(), bc.tobytes())
    if key not in _CACHE:
        barr, bindex = _build_bmats(Wc.astype(np.float64))
        # SBUF layout [p, n*128+m]
        if _MM_DTYPE == "bf16":
            parts = [barr.transpose(1, 0, 2).reshape(128, _NB * 128)]
            w0 = _NB * 128
        else:
            parts = [barr.transpose(1, 0, 2).reshape(128, _NB * 128), _build_masks()]
            w0 = _NB * 128 + NTILES
        if _PAD_BMAT:
            wpad = (w0 + 127) // 128 * 128 - w0
            if wpad:
                parts.append(np.zeros((128, wpad), np.float32))
        bflat = np.ascontiguousarray(np.concatenate(parts, axis=1), dtype=np.float32)
        if _MM_DTYPE == "bf16":
            import ml_dtypes

            bflat = bflat.astype(ml_dtypes.bfloat16)
        nc = _build_nc(bc.astype(np.float64), bindex)
        _CACHE[key] = (nc, bflat)
    return _CACHE[key]


def _install_trace_shim():
    """The agent image lacks antenv.axon_hooks; rebuild the NTFF hook from
    trn_boot's ctypes recipe and skip the artifact upload."""
    import types

    if "antenv.axon_hooks" in sys.modules:
        return
    try:
        from trn_agent_boot.trn_boot import _ntff_profile_via_ctypes

        hook = _ntff_profile_via_ctypes("/opt/axon/libaxon_pjrt.so")
    except Exception:
        hook = None
    mod = types.ModuleType("antenv.axon_hooks")
    mod.get_axon_ntff_profile_hook = lambda: hook
    mod.set_axon_ntff_profile_hook = lambda h: None
    sys.modules["antenv.axon_hooks"] = mod
    import concourse.bass_utils as bu

    bu.upload_artifacts = lambda d: "local://skipped"


def kernel(x, W, b, _trace=False, _tracedir=None):
    x = np.asarray(x)
    W = np.asarray(W)
    b = np.asarray(b)
    nc, bflat = _get_program(W, b)
    in_maps = []
    for c in range(NCORES):
        shard = np.ascontiguousarray(x[c * IMGS : (c + 1) * IMGS, 0]).astype(np.float32)
        im_map = {"xs": shard, "bmat": bflat}
        if _MM_DTYPE == "bf16":
            am = np.zeros((128, 16), np.float32)
            am[:, :NTILES] = _build_masks()
            for t in range(T_ITERS):
                for pair in range(2):
                    am[0:48, 5 + t * 2 + pair] = float(b[t, 2 * pair])
                    am[64:112, 5 + t * 2 + pair] = float(b[t, 2 * pair + 1])
            im_map["aux"] = am
        in_maps.append(im_map)
    kw = {}
    if _trace:
        _install_trace_shim()
        kw = {"trace": True, "tmpdir": _tracedir}
    res = run_bass_kernel_spmd(nc, in_maps, list(range(NCORES)), **kw)
    out = np.concatenate([res.results[c]["out"] for c in range(NCORES)], axis=0)
    out = out[:, None].astype(x.dtype)
    kernel._last = res
    return out

